# revision 4
# baseline (speedup 1.0000x reference)
"""EntityAwareAttention Trainium2 kernel.

Single-head attention (B=4, S=4096, H=768) with a per-key-column additive
entity bias and key mask:

    q = x @ Wq.T + bq ; k = x @ Wk.T + bk ; v = x @ Wv.T + bv
    scores = q @ k.T / sqrt(H) + col_add[None, :]      (col_add per key column)
    ctx = softmax(scores) @ v

Sharding: 8 cores = 4 batches x 2 query-halves.  Each core gets its batch's
full X.T (bf16) with the sequence columns rotated so that its 2048 queries are
always columns 0:2048 -> one SPMD program for all cores (key order permutation
is softmax-invariant as long as col_add is permuted identically).

Device layout trick: scores are computed TRANSPOSED (S.T[k, q], k on
partitions).  Then:
  * the per-key bias/mask is a per-partition activation bias fused into Exp
  * P.T = exp(S.T) is directly the rhs of the P@V matmul (ctx.T = V.T@P.T
    with V tiles as stationary) -> zero transposes anywhere
  * softmax normalizer l = column-sum of P.T via a ones[128,1] matmul,
    1/l = Exp(-Ln(l)) on the scalar engine, broadcast across partitions with
    a K=1 matmul, applied during PSUM->SBUF evacuation of ctx.T
  * max-subtraction is skipped: scores are O(1)-bounded (bias <= ~64), exp
    cannot overflow fp32, and softmax is shift-invariant.
"""

import math

import numpy as np
import ml_dtypes

import concourse.bass as bass
import concourse.bacc as bacc
import concourse.tile as tile
from concourse import mybir
from concourse.bass import ts
from concourse.bass_utils import run_bass_kernel_spmd

P = 128
F32 = mybir.dt.float32
BF16 = mybir.dt.bfloat16
AF = mybir.ActivationFunctionType


def build_attention_bass(S, H, QH, QC=512, bv_nonzero=True):
    """One NeuronCore's program: full keys S, queries = first QH rotated cols."""
    HT = H // P           # h/o tiles
    KT = S // P           # key tiles
    NQC = QH // QC        # query chunks
    NKC = S // 512        # key chunks for K.T projection
    # V projection free-dim chunks (<=512 each, equal-ish)
    nvc = math.ceil(H / 512)
    VC = H // nvc
    assert H % nvc == 0 and VC <= 512

    nc = bacc.Bacc(trn_type="TRN2")

    xt_d = nc.dram_tensor("xt", [HT, P, S], BF16, kind="ExternalInput")
    wqt_d = nc.dram_tensor("wqt", [HT, P, H], BF16, kind="ExternalInput")
    wkt_d = nc.dram_tensor("wkt", [HT, P, H], BF16, kind="ExternalInput")
    wvt_d = nc.dram_tensor("wvt", [HT, P, H], BF16, kind="ExternalInput")
    bq_d = nc.dram_tensor("bq2", [P, HT], F32, kind="ExternalInput")
    bk_d = nc.dram_tensor("bk2", [P, HT], F32, kind="ExternalInput")
    bv_d = nc.dram_tensor("bv2", [P, HT], F32, kind="ExternalInput")
    col_d = nc.dram_tensor("col", [P, KT], F32, kind="ExternalInput")
    out_d = nc.dram_tensor("out", [HT, P, QH], F32, kind="ExternalOutput")

    with tile.TileContext(nc) as tc:
        with (
            tc.tile_pool(name="persist", bufs=1) as persist,
            tc.tile_pool(name="small", bufs=1) as small,
        ):
            kt_sb = persist.tile([P, HT, S], BF16, tag="kt")
            v_sb = persist.tile([P, KT, H], BF16, tag="v")
            qt_sb = persist.tile([P, HT, QH], BF16, tag="qt")

            colb = small.tile([P, KT], F32, tag="colb")
            nc.sync.dma_start(colb, col_d[:, :])
            bq_sb = small.tile([P, HT], F32, tag="bq_sb")
            nc.sync.dma_start(bq_sb, bq_d[:, :])
            bk_sb = small.tile([P, HT], F32, tag="bk_sb")
            nc.sync.dma_start(bk_sb, bk_d[:, :])
            bv_sb = small.tile([P, HT], F32, tag="bv_sb")
            nc.sync.dma_start(bv_sb, bv_d[:, :])
            ones_col = small.tile([P, 1], BF16, tag="ones_col")
            nc.vector.memset(ones_col, 1.0)
            ones_row = small.tile([1, P], F32, tag="ones_row")
            nc.vector.memset(ones_row, 1.0)

            # ---------------- Phase 1: projections ----------------
            with (
                tc.tile_pool(name="xw", bufs=1) as xpool,
                tc.tile_pool(name="wpool", bufs=2) as wpool,
                tc.tile_pool(name="ppsum", bufs=3, space="PSUM") as ppsum,
            ):
                xt_sb = xpool.tile([P, HT, S], BF16, tag="xt_sb")
                for ht in range(HT):
                    nc.sync.dma_start(xt_sb[:, ht, :], xt_d[ht, :, :])

                # K.T[o, k] (o on partitions), bias bk fused into evacuation
                wk_sb = wpool.tile([P, HT, H], BF16, tag="w", name="wk_sb")
                for ht in range(HT):
                    nc.sync.dma_start(wk_sb[:, ht, :], wkt_d[ht, :, :])
                for ot in range(HT):
                    for kc in range(NKC):
                        pps = ppsum.tile([P, 512], F32, tag="pp", name="pps")
                        for ht in range(HT):
                            nc.tensor.matmul(
                                pps,
                                wk_sb[:, ht, ts(ot, P)],
                                xt_sb[:, ht, ts(kc, 512)],
                                start=(ht == 0),
                                stop=(ht == HT - 1),
                            )
                        nc.scalar.activation(
                            kt_sb[:, ot, ts(kc, 512)], pps, AF.Identity,
                            bias=bk_sb[:, ot : ot + 1], scale=1.0,
                        )

                # Q.T[o, q] for this core's queries (= rotated cols 0:QH)
                wq_sb = wpool.tile([P, HT, H], BF16, tag="w", name="wq_sb")
                for ht in range(HT):
                    nc.sync.dma_start(wq_sb[:, ht, :], wqt_d[ht, :, :])
                for ot in range(HT):
                    for qc in range(QH // 512):
                        pps = ppsum.tile([P, 512], F32, tag="pp", name="pps")
                        for ht in range(HT):
                            nc.tensor.matmul(
                                pps,
                                wq_sb[:, ht, ts(ot, P)],
                                xt_sb[:, ht, ts(qc, 512)],
                                start=(ht == 0),
                                stop=(ht == HT - 1),
                            )
                        nc.scalar.activation(
                            qt_sb[:, ot, ts(qc, 512)], pps, AF.Identity,
                            bias=bq_sb[:, ot : ot + 1], scale=1.0,
                        )

                # V[s, o] (s on partitions) - no bias here; bv is applied to
                # the final context (softmax rows sum to 1).
                wv_sb = wpool.tile([P, HT, H], BF16, tag="w", name="wv_sb")
                for ht in range(HT):
                    nc.sync.dma_start(wv_sb[:, ht, :], wvt_d[ht, :, :])
                for st in range(KT):
                    for oc in range(nvc):
                        ppv = ppsum.tile([P, VC], F32, tag="ppv", name="ppv")
                        for ht in range(HT):
                            nc.tensor.matmul(
                                ppv,
                                xt_sb[:, ht, ts(st, P)],
                                wv_sb[:, ht, ts(oc, VC)],
                                start=(ht == 0),
                                stop=(ht == HT - 1),
                            )
                        nc.any.tensor_copy(v_sb[:, st, ts(oc, VC)], ppv)

            # ---------------- Phase 2: attention ----------------
            with (
                tc.tile_pool(name="ptp", bufs=1) as ptp,
                tc.tile_pool(name="stp", bufs=2, space="PSUM") as stp,
                tc.tile_pool(name="ctxp", bufs=2, space="PSUM") as ctxp,
                tc.tile_pool(name="lp", bufs=1, space="PSUM") as lp,
                tc.tile_pool(name="osb", bufs=3) as osb,
                tc.tile_pool(name="lsb", bufs=2) as lsb,
            ):
                for qc in range(NQC):
                    pt = ptp.tile([P, KT, QC], BF16, tag="pt", name="pt")
                    # scores S.T[k, qchunk] tile-by-tile, exp fused w/ col bias
                    for kt in range(KT):
                        st_ps = stp.tile([P, QC], F32, tag="st", name="st_ps")
                        for ot in range(HT):
                            nc.tensor.matmul(
                                st_ps,
                                kt_sb[:, ot, ts(kt, P)],
                                qt_sb[:, ot, ts(qc, QC)],
                                start=(ot == 0),
                                stop=(ot == HT - 1),
                            )
                        nc.scalar.activation(
                            pt[:, kt, :], st_ps, AF.Exp,
                            bias=colb[:, kt : kt + 1], scale=1.0,
                        )
                    # l[q] = sum_k P.T[k, q]
                    l_ps = lp.tile([1, QC], F32, tag="l", name="l_ps")
                    for kt in range(KT):
                        nc.tensor.matmul(
                            l_ps, ones_col, pt[:, kt, :],
                            start=(kt == 0), stop=(kt == KT - 1),
                        )
                    # r = 1/l = Exp(-Ln(l)); same ACT table set as Exp
                    lnl = lsb.tile([1, QC], F32, tag="lnl", name="lnl")
                    nc.scalar.activation(lnl, l_ps, AF.Ln, scale=1.0)
                    r_sb = lsb.tile([1, QC], F32, tag="r", name="r_sb")
                    nc.scalar.activation(r_sb, lnl, AF.Exp, scale=-1.0)
                    # broadcast r across partitions with a K=1 matmul
                    bc_ps = lp.tile([P, QC], F32, tag="bc", name="bc_ps")
                    nc.tensor.matmul(bc_ps, ones_row, r_sb, start=True, stop=True)
                    bc_sb = lsb.tile([P, QC], F32, tag="bc_sb", name="bc_sb")
                    nc.any.tensor_copy(bc_sb, bc_ps)
                    # ctx.T[o, qchunk] = V.T @ P.T ; normalize on evacuation
                    for ot in range(HT):
                        cps = ctxp.tile([P, QC], F32, tag="ctx", name="cps")
                        for kt in range(KT):
                            nc.tensor.matmul(
                                cps,
                                v_sb[:, kt, ts(ot, P)],
                                pt[:, kt, :],
                                start=(kt == 0),
                                stop=(kt == KT - 1),
                            )
                        o_sb = osb.tile([P, QC], F32, tag="o", name="o_sb")
                        nc.vector.tensor_tensor(
                            o_sb, cps, bc_sb, mybir.AluOpType.mult
                        )
                        if bv_nonzero:
                            nc.vector.tensor_scalar_add(
                                o_sb, o_sb, bv_sb[:, ot : ot + 1]
                            )
                        nc.sync.dma_start(out_d[ot, :, ts(qc, QC)], o_sb)
    nc.finalize()
    return nc


# ------------------------- host side -------------------------

_NC_CACHE = {}
TRACE = False
_LAST_RESULTS = None


def _get_nc(S, H, QH, bv_nonzero):
    key = (S, H, QH, bv_nonzero)
    if key not in _NC_CACHE:
        _NC_CACHE[key] = build_attention_bass(S, H, QH, bv_nonzero=bv_nonzero)
    return _NC_CACHE[key]


def _prep_core_inputs(hs_b, col_b, off, S, H, shared):
    """Per-core input dict: rotated X.T (bf16) + rotated col_add."""
    HT = H // P
    KT = S // P
    xt = np.ascontiguousarray(hs_b.T)  # [H, S] f32
    if off:
        xt = np.concatenate([xt[:, off:], xt[:, :off]], axis=1)
    xt = xt.astype(ml_dtypes.bfloat16).reshape(HT, P, S)
    col = np.roll(col_b, -off) if off else col_b
    col = np.ascontiguousarray(col.reshape(KT, P).T, dtype=np.float32)
    d = {"xt": xt, "col": col}
    d.update(shared)
    return d


def kernel(hidden_states, attention_mask, entity_positions, Wq, bq, Wk, bk, Wv, bv):
    hs = np.asarray(hidden_states, dtype=np.float32)
    am = np.asarray(attention_mask, dtype=np.float32)
    ep = np.asarray(entity_positions)
    Wq = np.asarray(Wq, dtype=np.float32)
    Wk = np.asarray(Wk, dtype=np.float32)
    Wv = np.asarray(Wv, dtype=np.float32)
    bq = np.asarray(bq, dtype=np.float32)
    bk = np.asarray(bk, dtype=np.float32)
    bv = np.asarray(bv, dtype=np.float32)

    B, S, H = hs.shape
    QH = S // 2
    HT = H // P
    scale = 1.0 / math.sqrt(H)

    # per-key-column additive term: entity bias (+1 per entity occurrence,
    # duplicates accumulate) + mask
    bias_cols = np.zeros((B, S), dtype=np.float32)
    np.add.at(bias_cols, (np.arange(B)[:, None], ep.astype(np.int64)), 1.0)
    col_add = bias_cols + (1.0 - am) * (-10000.0)

    def wtile(w, s):
        # [o, h] -> scaled transpose [h, o] -> [HT, P, H] bf16
        return np.ascontiguousarray(w.T * s).astype(ml_dtypes.bfloat16).reshape(HT, P, H)

    def btile(b, s):
        # [o] -> [P, HT] with [p, t] = b[t*P + p]
        return np.ascontiguousarray((b * s).reshape(HT, P).T, dtype=np.float32)

    shared = {
        "wqt": wtile(Wq, scale),
        "wkt": wtile(Wk, 1.0),
        "wvt": wtile(Wv, 1.0),
        "bq2": btile(bq, scale),
        "bk2": btile(bk, 1.0),
        "bv2": btile(bv, 1.0),
    }

    n_cores = 2 * B
    in_maps = []
    for core in range(n_cores):
        b, half = core // 2, core % 2
        in_maps.append(
            _prep_core_inputs(hs[b], col_add[b], half * QH, S, H, shared)
        )

    nc = _get_nc(S, H, QH, bool(np.any(bv != 0.0)))
    kw = {}
    if TRACE:
        kw = dict(trace=True, trace_cores=[0])
    res = run_bass_kernel_spmd(nc, in_maps, core_ids=list(range(n_cores)), **kw)
    global _LAST_RESULTS
    _LAST_RESULTS = res

    out = np.empty((B, S, H), dtype=np.float32)
    for core in range(n_cores):
        b, half = core // 2, core % 2
        ctx_t = res.results[core]["out"].reshape(H, QH)  # [o, q]
        out[b, half * QH : (half + 1) * QH, :] = ctx_t.T
    return out


# revision 5
# speedup vs baseline: 1.0508x; 1.0508x over previous
"""EntityAwareAttention Trainium2 kernel.

Single-head attention (B=4, S=4096, H=768) with a per-key-column additive
entity bias and key mask:

    q = x @ Wq.T + bq ; k = x @ Wk.T + bk ; v = x @ Wv.T + bv
    scores = q @ k.T / sqrt(H) + col_add[None, :]      (col_add per key column)
    ctx = softmax(scores) @ v

Sharding: 8 cores = 4 batches x 2 query-halves.  Each core gets its batch's
full X.T (bf16) with the sequence columns rotated so that its 2048 queries are
always columns 0:2048 -> one SPMD program for all cores (key order permutation
is softmax-invariant as long as col_add is permuted identically).

Device layout trick: scores are computed TRANSPOSED (S.T[k, q], k on
partitions).  Then:
  * the per-key bias/mask is a per-partition activation bias fused into Exp
  * P.T = exp(S.T) is directly the rhs of the P@V matmul (ctx.T = V.T@P.T
    with V tiles as stationary) -> zero transposes anywhere
  * softmax normalizer l = column-sum of P.T via a ones[128,1] matmul,
    1/l = Exp(-Ln(l)) on the scalar engine, broadcast across partitions with
    a K=1 matmul, applied during PSUM->SBUF evacuation of ctx.T
  * max-subtraction is skipped: scores are O(1)-bounded (bias <= ~64), exp
    cannot overflow fp32, and softmax is shift-invariant.
"""

import math

import numpy as np
import ml_dtypes

import concourse.bass as bass
import concourse.bacc as bacc
import concourse.tile as tile
from concourse import mybir
from concourse.bass import ts
from concourse.bass_utils import run_bass_kernel_spmd

P = 128
F32 = mybir.dt.float32
BF16 = mybir.dt.bfloat16
AF = mybir.ActivationFunctionType


def build_attention_bass(S, H, QH, QC=512, bv_nonzero=True):
    """One NeuronCore's program: full keys S, queries = first QH rotated cols."""
    HT = H // P           # h/o tiles
    KT = S // P           # key tiles
    NQC = QH // QC        # query chunks
    NKC = S // 512        # key chunks for K.T projection
    # V projection free-dim chunks (<=512 each, equal-ish)
    nvc = math.ceil(H / 512)
    VC = H // nvc
    assert H % nvc == 0 and VC <= 512

    nc = bacc.Bacc(trn_type="TRN2")

    xt_d = nc.dram_tensor("xt", [HT, P, S], BF16, kind="ExternalInput")
    wqt_d = nc.dram_tensor("wqt", [HT, P, H], BF16, kind="ExternalInput")
    wkt_d = nc.dram_tensor("wkt", [HT, P, H], BF16, kind="ExternalInput")
    wvt_d = nc.dram_tensor("wvt", [HT, P, H], BF16, kind="ExternalInput")
    bq_d = nc.dram_tensor("bq2", [P, HT], F32, kind="ExternalInput")
    bk_d = nc.dram_tensor("bk2", [P, HT], F32, kind="ExternalInput")
    bv_d = nc.dram_tensor("bv2", [P, HT], F32, kind="ExternalInput")
    col_d = nc.dram_tensor("col", [P, KT], F32, kind="ExternalInput")
    out_d = nc.dram_tensor("out", [HT, P, QH], F32, kind="ExternalOutput")

    with tile.TileContext(nc) as tc:
        with (
            tc.tile_pool(name="persist", bufs=1) as persist,
            tc.tile_pool(name="small", bufs=1) as small,
        ):
            kt_sb = persist.tile([P, HT, S], BF16, tag="kt")
            v_sb = persist.tile([P, KT, H], BF16, tag="v")
            qt_sb = persist.tile([P, HT, QH], BF16, tag="qt")

            colb = small.tile([P, KT], F32, tag="colb")
            nc.sync.dma_start(colb, col_d[:, :])
            bq_sb = small.tile([P, HT], F32, tag="bq_sb")
            nc.sync.dma_start(bq_sb, bq_d[:, :])
            bk_sb = small.tile([P, HT], F32, tag="bk_sb")
            nc.sync.dma_start(bk_sb, bk_d[:, :])
            bv_sb = small.tile([P, HT], F32, tag="bv_sb")
            nc.sync.dma_start(bv_sb, bv_d[:, :])
            ones_col = small.tile([P, 1], F32, tag="ones_col")
            nc.vector.memset(ones_col, 1.0)
            ones_row = small.tile([1, P], F32, tag="ones_row")
            nc.vector.memset(ones_row, 1.0)

            # ---------------- Phase 1: projections ----------------
            with (
                tc.tile_pool(name="xw", bufs=1) as xpool,
                tc.tile_pool(name="wpool", bufs=2) as wpool,
                tc.tile_pool(name="ppsum", bufs=3, space="PSUM") as ppsum,
            ):
                xt_sb = xpool.tile([P, HT, S], BF16, tag="xt_sb")

                # Q weights + the query half of X.T first: the PE can start on
                # Q.T while the rest of X.T is still in flight.
                wq_sb = wpool.tile([P, HT, H], BF16, tag="w", name="wq_sb")
                for ht in range(HT):
                    nc.sync.dma_start(wq_sb[:, ht, :], wqt_d[ht, :, :])
                XCH = 512  # xt dma chunk (columns) - spread across DMA queues
                for ht in range(HT):
                    for xc in range(QH // XCH):
                        nc.sync.dma_start(
                            xt_sb[:, ht, ts(xc, XCH)], xt_d[ht, :, ts(xc, XCH)]
                        )
                wk_sb = wpool.tile([P, HT, H], BF16, tag="w", name="wk_sb")
                for ht in range(HT):
                    nc.sync.dma_start(wk_sb[:, ht, :], wkt_d[ht, :, :])
                for ht in range(HT):
                    for xc in range(QH // XCH, S // XCH):
                        nc.sync.dma_start(
                            xt_sb[:, ht, ts(xc, XCH)], xt_d[ht, :, ts(xc, XCH)]
                        )

                # Q.T[o, q] for this core's queries (= rotated cols 0:QH)
                for ot in range(HT):
                    for qc in range(QH // 512):
                        pps = ppsum.tile([P, 512], F32, tag="pp", name="pps")
                        for ht in range(HT):
                            nc.tensor.matmul(
                                pps,
                                wq_sb[:, ht, ts(ot, P)],
                                xt_sb[:, ht, ts(qc, 512)],
                                start=(ht == 0),
                                stop=(ht == HT - 1),
                            )
                        nc.scalar.activation(
                            qt_sb[:, ot, ts(qc, 512)], pps, AF.Identity,
                            bias=bq_sb[:, ot : ot + 1], scale=1.0,
                        )

                # K.T[o, k] (o on partitions), bias bk fused into evacuation.
                # Column chunks ordered so the first-half ones (already
                # resident) run while the second half of X.T lands.
                for kc in range(NKC):
                    for ot in range(HT):
                        pps = ppsum.tile([P, 512], F32, tag="pp", name="pps")
                        for ht in range(HT):
                            nc.tensor.matmul(
                                pps,
                                wk_sb[:, ht, ts(ot, P)],
                                xt_sb[:, ht, ts(kc, 512)],
                                start=(ht == 0),
                                stop=(ht == HT - 1),
                            )
                        nc.scalar.activation(
                            kt_sb[:, ot, ts(kc, 512)], pps, AF.Identity,
                            bias=bk_sb[:, ot : ot + 1], scale=1.0,
                        )

                # V[s, o] (s on partitions) - no bias here; bv is applied to
                # the final context (softmax rows sum to 1).
                wv_sb = wpool.tile([P, HT, H], BF16, tag="w", name="wv_sb")
                for ht in range(HT):
                    nc.sync.dma_start(wv_sb[:, ht, :], wvt_d[ht, :, :])
                for st in range(KT):
                    for oc in range(nvc):
                        ppv = ppsum.tile([P, VC], F32, tag="ppv", name="ppv")
                        for ht in range(HT):
                            nc.tensor.matmul(
                                ppv,
                                xt_sb[:, ht, ts(st, P)],
                                wv_sb[:, ht, ts(oc, VC)],
                                start=(ht == 0),
                                stop=(ht == HT - 1),
                            )
                        nc.any.tensor_copy(v_sb[:, st, ts(oc, VC)], ppv)

            # ---------------- Phase 2: attention ----------------
            with (
                tc.tile_pool(name="ptp", bufs=1) as ptp,
                tc.tile_pool(name="stp", bufs=2, space="PSUM") as stp,
                tc.tile_pool(name="ctxp", bufs=2, space="PSUM") as ctxp,
                tc.tile_pool(name="lp", bufs=1, space="PSUM") as lp,
                tc.tile_pool(name="osb", bufs=3) as osb,
                tc.tile_pool(name="lsb", bufs=2) as lsb,
            ):
                for qc in range(NQC):
                    pt = ptp.tile([P, KT, QC], BF16, tag="pt", name="pt")
                    # scores S.T[k, qchunk] tile-by-tile, exp fused w/ col bias
                    for kt in range(KT):
                        st_ps = stp.tile([P, QC], F32, tag="st", name="st_ps")
                        for ot in range(HT):
                            nc.tensor.matmul(
                                st_ps,
                                kt_sb[:, ot, ts(kt, P)],
                                qt_sb[:, ot, ts(qc, QC)],
                                start=(ot == 0),
                                stop=(ot == HT - 1),
                            )
                        nc.scalar.activation(
                            pt[:, kt, :], st_ps, AF.Exp,
                            bias=colb[:, kt : kt + 1], scale=1.0,
                        )
                    # l[q] = sum_k P.T[k, q]: partial sums on the (idle)
                    # vector engine, then one 128->1 matmul reduction.
                    lacc = lsb.tile([P, QC], F32, tag="lacc", name="lacc")
                    nc.vector.tensor_copy(lacc, pt[:, 0, :])
                    for kt in range(1, KT):
                        nc.vector.tensor_tensor(
                            lacc, lacc, pt[:, kt, :], mybir.AluOpType.add
                        )
                    l_ps = lp.tile([1, QC], F32, tag="l", name="l_ps")
                    nc.tensor.matmul(l_ps, ones_col, lacc, start=True, stop=True)
                    # r = 1/l = Exp(-Ln(l)); same ACT table set as Exp
                    lnl = lsb.tile([1, QC], F32, tag="lnl", name="lnl")
                    nc.scalar.activation(lnl, l_ps, AF.Ln, scale=1.0)
                    r_sb = lsb.tile([1, QC], F32, tag="r", name="r_sb")
                    nc.scalar.activation(r_sb, lnl, AF.Exp, scale=-1.0)
                    # broadcast r across partitions with a K=1 matmul
                    bc_ps = lp.tile([P, QC], F32, tag="bc", name="bc_ps")
                    nc.tensor.matmul(bc_ps, ones_row, r_sb, start=True, stop=True)
                    bc_sb = lsb.tile([P, QC], F32, tag="bc_sb", name="bc_sb")
                    nc.any.tensor_copy(bc_sb, bc_ps)
                    # ctx.T[o, qchunk] = V.T @ P.T ; normalize on evacuation
                    for ot in range(HT):
                        cps = ctxp.tile([P, QC], F32, tag="ctx", name="cps")
                        for kt in range(KT):
                            nc.tensor.matmul(
                                cps,
                                v_sb[:, kt, ts(ot, P)],
                                pt[:, kt, :],
                                start=(kt == 0),
                                stop=(kt == KT - 1),
                            )
                        o_sb = osb.tile([P, QC], F32, tag="o", name="o_sb")
                        nc.vector.tensor_tensor(
                            o_sb, cps, bc_sb, mybir.AluOpType.mult
                        )
                        if bv_nonzero:
                            nc.vector.tensor_scalar_add(
                                o_sb, o_sb, bv_sb[:, ot : ot + 1]
                            )
                        nc.sync.dma_start(out_d[ot, :, ts(qc, QC)], o_sb)
    nc.finalize()
    return nc


# ------------------------- host side -------------------------

_NC_CACHE = {}
TRACE = False
_LAST_RESULTS = None


def _get_nc(S, H, QH, bv_nonzero):
    key = (S, H, QH, bv_nonzero)
    if key not in _NC_CACHE:
        _NC_CACHE[key] = build_attention_bass(S, H, QH, bv_nonzero=bv_nonzero)
    return _NC_CACHE[key]


def _prep_core_inputs(hs_b, col_b, off, S, H, shared):
    """Per-core input dict: rotated X.T (bf16) + rotated col_add."""
    HT = H // P
    KT = S // P
    xt = np.ascontiguousarray(hs_b.T)  # [H, S] f32
    if off:
        xt = np.concatenate([xt[:, off:], xt[:, :off]], axis=1)
    xt = xt.astype(ml_dtypes.bfloat16).reshape(HT, P, S)
    col = np.roll(col_b, -off) if off else col_b
    col = np.ascontiguousarray(col.reshape(KT, P).T, dtype=np.float32)
    d = {"xt": xt, "col": col}
    d.update(shared)
    return d


def kernel(hidden_states, attention_mask, entity_positions, Wq, bq, Wk, bk, Wv, bv):
    hs = np.asarray(hidden_states, dtype=np.float32)
    am = np.asarray(attention_mask, dtype=np.float32)
    ep = np.asarray(entity_positions)
    Wq = np.asarray(Wq, dtype=np.float32)
    Wk = np.asarray(Wk, dtype=np.float32)
    Wv = np.asarray(Wv, dtype=np.float32)
    bq = np.asarray(bq, dtype=np.float32)
    bk = np.asarray(bk, dtype=np.float32)
    bv = np.asarray(bv, dtype=np.float32)

    B, S, H = hs.shape
    QH = S // 2
    HT = H // P
    scale = 1.0 / math.sqrt(H)

    # per-key-column additive term: entity bias (+1 per entity occurrence,
    # duplicates accumulate) + mask
    bias_cols = np.zeros((B, S), dtype=np.float32)
    np.add.at(bias_cols, (np.arange(B)[:, None], ep.astype(np.int64)), 1.0)
    col_add = bias_cols + (1.0 - am) * (-10000.0)

    def wtile(w, s):
        # [o, h] -> scaled transpose [h, o] -> [HT, P, H] bf16
        return np.ascontiguousarray(w.T * s).astype(ml_dtypes.bfloat16).reshape(HT, P, H)

    def btile(b, s):
        # [o] -> [P, HT] with [p, t] = b[t*P + p]
        return np.ascontiguousarray((b * s).reshape(HT, P).T, dtype=np.float32)

    shared = {
        "wqt": wtile(Wq, scale),
        "wkt": wtile(Wk, 1.0),
        "wvt": wtile(Wv, 1.0),
        "bq2": btile(bq, scale),
        "bk2": btile(bk, 1.0),
        "bv2": btile(bv, 1.0),
    }

    n_cores = 2 * B
    in_maps = []
    for core in range(n_cores):
        b, half = core // 2, core % 2
        in_maps.append(
            _prep_core_inputs(hs[b], col_add[b], half * QH, S, H, shared)
        )

    nc = _get_nc(S, H, QH, bool(np.any(bv != 0.0)))
    kw = {}
    if TRACE:
        kw = dict(trace=True, trace_cores=[0])
    res = run_bass_kernel_spmd(nc, in_maps, core_ids=list(range(n_cores)), **kw)
    global _LAST_RESULTS
    _LAST_RESULTS = res

    out = np.empty((B, S, H), dtype=np.float32)
    for core in range(n_cores):
        b, half = core // 2, core % 2
        ctx_t = res.results[core]["out"].reshape(H, QH)  # [o, q]
        out[b, half * QH : (half + 1) * QH, :] = ctx_t.T
    return out
